# revision 5
# baseline (speedup 1.0000x reference)
"""Multi-head causal attention (B=4, S=2048, H=16, D=64) on 8 trn2 cores.

Sharding: core c -> (batch b = c//2, half = c%2). Each core computes the
full K/V projections for its batch and attention + output projection for
a zigzag set of 4 query chunks of 256 rows ({0,3,4,7} or {1,2,5,6}) so
that the causal-attention work per core is identical in structure
(uniform SPMD program); per-core differences are carried entirely by the
input data (query rows, additive masks). No collectives needed: each
core owns disjoint output rows.
"""

import numpy as np
import ml_dtypes

import concourse.bass as bass
import concourse.mybir as mybir
from concourse.tile import TileContext
from concourse.bass_utils import run_bass_kernel_spmd

F32 = mybir.dt.float32
BF16 = mybir.dt.bfloat16

B, S, H, D = 4, 2048, 16, 64
DM = H * D           # 1024
QCH = 256            # query chunk rows
NCH = S // QCH       # 8 global chunks per batch
QROWS = S // 2       # query rows per core (1024)
KB = 128             # k block rows
SCHED = (4, 8, 12, 16)   # padded k-extent (in KB blocks) per local slot
GMAP = ((0, 3, 4, 7), (1, 2, 5, 6))  # global chunk per (half, slot)
MASK_NEG = -30000.0

N_CORES = 8


def _split_excess_waits(nc):
    """walrus on this stack accepts at most ONE semaphore wait per
    instruction; Tile emits more on drains/branches/etc. Move excess
    waits onto preceding same-engine nops (semantically identical: the
    engine blocks on the nops first)."""
    for f in nc.m.functions:
        for bb in f.blocks:
            new_instrs = []
            for ins in bb.instructions:
                si = ins.sync_info
                if si is not None and si.on_wait is not None and len(si.on_wait) > 1:
                    waits = list(si.on_wait)
                    extra, keep = waits[:-1], waits[-1:]
                    for i, w in enumerate(extra):
                        new_instrs.append(mybir.InstNoOp(
                            name=f"{ins.name}-ws{i}", engine=ins.engine,
                            ins=[], outs=[],
                            sync_info=mybir.SyncInfo(on_wait=[w], on_update=[])))
                    ins.sync_info = mybir.SyncInfo(on_wait=keep,
                                                   on_update=list(si.on_update))
                new_instrs.append(ins)
            bb.instructions[:] = new_instrs


def _load_transposed(nc, pool, src_dram, rows, name):
    """Load src [rows, DM] f32 from DRAM -> bf16 transposed SBUF tiles:
    returns 8 tiles [128, rows] (tile j = dm rows 128j..128j+127)."""
    tts = [pool.tile([128, rows], BF16, tag=f"{name}T{j}", name=f"{name}T{j}") for j in range(8)]
    n_rt = rows // 128
    for st in range(n_rt):
        raw = pool.tile([128, DM], F32, tag=f"{name}_raw", bufs=3)
        nc.sync.dma_start(raw[:], src_dram[st * 128:(st + 1) * 128, :])
        bfv = pool.tile([128, DM], BF16, tag=f"{name}_bf", bufs=3)
        nc.vector.tensor_copy(bfv[:], raw[:])
        for j in range(8):
            nc.sync.dma_start_transpose(
                tts[j][:, st * 128:(st + 1) * 128],
                bfv[:, j * 128:(j + 1) * 128])
    return tts


def build_mha(masking: bool):
    nc = bass.Bass()

    q_in = nc.dram_tensor("q_in", [QROWS, DM], F32, kind="ExternalInput")
    k_in = nc.dram_tensor("k_in", [S, DM], F32, kind="ExternalInput")
    v_in = nc.dram_tensor("v_in", [S, DM], F32, kind="ExternalInput")
    wqt = nc.dram_tensor("wqt", [DM, DM], BF16, kind="ExternalInput")
    wkt = nc.dram_tensor("wkt", [DM, DM], BF16, kind="ExternalInput")
    wvt = nc.dram_tensor("wvt", [DM, DM], BF16, kind="ExternalInput")
    wot = nc.dram_tensor("wot", [DM, DM], BF16, kind="ExternalInput")
    bq2 = nc.dram_tensor("bq2", [128, 8], F32, kind="ExternalInput")
    bk2 = nc.dram_tensor("bk2", [128, 8], F32, kind="ExternalInput")
    bvr = nc.dram_tensor("bvr", [1, DM], BF16, kind="ExternalInput")
    bor = nc.dram_tensor("bor", [1, DM], BF16, kind="ExternalInput")
    msk = nc.dram_tensor("msk", [4, 128, 1024], F32, kind="ExternalInput")
    out = nc.dram_tensor("out", [QROWS, DM], F32, kind="ExternalOutput")

    sched = SCHED if masking else (16, 16, 16, 16)

    with TileContext(nc) as tc:
        with (
            tc.tile_pool(name="persist", bufs=1) as pp,
            tc.tile_pool(name="psA", bufs=2, space="PSUM") as psA,
        ):
            # ---- constants ----
            bq_sb = pp.tile([128, 8], F32, tag="bq")
            bk_sb = pp.tile([128, 8], F32, tag="bk")
            nc.sync.dma_start(bq_sb[:], bq2[:])
            nc.sync.dma_start(bk_sb[:], bk2[:])
            bv_sb = pp.tile([1, DM], BF16, tag="bv")
            nc.sync.dma_start(bv_sb[:], bvr[:])
            bo_sb = pp.tile([1, DM], BF16, tag="bo")
            nc.sync.dma_start(bo_sb[:], bor[:])
            ones_sb = pp.tile([1, 128], BF16, tag="ones")
            nc.vector.memset(ones_sb[:], 1.0)
            mask_sb = []
            if masking:
                for s in range(4):
                    mt = pp.tile([128, 1024], F32, tag=f"msk{s}")
                    nc.sync.dma_start(mt[:], msk[s])
                    mask_sb.append(mt)

            # ---- persistent activation storage ----
            kT = [pp.tile([128, S], BF16, tag=f"kT{j}", name=f"kT{j}") for j in range(8)]
            qT = [pp.tile([128, QROWS], BF16, tag=f"qT{j}", name=f"qT{j}") for j in range(8)]
            v_sb = [pp.tile([128, H * (D + 1)], BF16, tag=f"v{t}", name=f"v{t}")
                    for t in range(S // 128)]

            # ---- K projection ----
            with tc.tile_pool(name="kstage", bufs=1) as ksp:
                keyT = _load_transposed(nc, ksp, k_in, S, "key")
                w_sb = [ksp.tile([128, DM], BF16, tag=f"wk{j}", name=f"wk{j}") for j in range(8)]
                for j in range(8):
                    nc.sync.dma_start(w_sb[j][:], wkt[j * 128:(j + 1) * 128, :])
                for i in range(8):
                    for sc in range(4):
                        p = psA.tile([128, 512], F32, tag="proj")
                        for j in range(8):
                            nc.tensor.matmul(
                                p[:], w_sb[j][:, i * 128:(i + 1) * 128],
                                keyT[j][:, sc * 512:(sc + 1) * 512],
                                start=(j == 0), stop=(j == 7))
                        nc.vector.tensor_scalar_add(
                            kT[i][:, sc * 512:(sc + 1) * 512], p[:],
                            bk_sb[:, i:i + 1])

            # ---- V projection (v_ext layout: per s-tile [128, 16*65],
            #      head h cols 65h..65h+63, col 65h+64 = ones) ----
            with tc.tile_pool(name="vstage", bufs=1) as vsp:
                valT = _load_transposed(nc, vsp, v_in, S, "val")
                w_sb = [vsp.tile([128, DM], BF16, tag=f"wv{j}", name=f"wv{j}") for j in range(8)]
                for j in range(8):
                    nc.sync.dma_start(w_sb[j][:], wvt[j * 128:(j + 1) * 128, :])
                for st in range(S // 128):
                    v3 = v_sb[st].rearrange("p (h x) -> p h x", x=D + 1)
                    nc.vector.memset(v3[:, :, 64:65], 1.0)
                    for c in range(2):
                        p = psA.tile([128, 512], F32, tag="proj")
                        for j in range(8):
                            nc.tensor.matmul(
                                p[:], valT[j][:, st * 128:(st + 1) * 128],
                                w_sb[j][:, c * 512:(c + 1) * 512],
                                start=(j == 0), stop=False)
                        nc.tensor.matmul(
                            p[:], ones_sb[:], bv_sb[:, c * 512:(c + 1) * 512],
                            start=False, stop=True)
                        nc.vector.tensor_copy(
                            v3[:, c * 8:(c + 1) * 8, 0:64], p[:])

            # ---- Q projection ----
            with tc.tile_pool(name="qstage", bufs=1) as qsp:
                quT = _load_transposed(nc, qsp, q_in, QROWS, "qu")
                w_sb = [qsp.tile([128, DM], BF16, tag=f"wq{j}", name=f"wq{j}") for j in range(8)]
                for j in range(8):
                    nc.sync.dma_start(w_sb[j][:], wqt[j * 128:(j + 1) * 128, :])
                for i in range(8):
                    for sc in range(2):
                        p = psA.tile([128, 512], F32, tag="proj")
                        for j in range(8):
                            nc.tensor.matmul(
                                p[:], w_sb[j][:, i * 128:(i + 1) * 128],
                                quT[j][:, sc * 512:(sc + 1) * 512],
                                start=(j == 0), stop=(j == 7))
                        nc.vector.tensor_scalar_add(
                            qT[i][:, sc * 512:(sc + 1) * 512], p[:],
                            bq_sb[:, i:i + 1])

            # ---- attention ----
            attn = [pp.tile([128, DM], BF16, tag=f"attn{t}", name=f"attn{t}")
                    for t in range(QROWS // 128)]
            with (
                tc.tile_pool(name="scores", bufs=2, space="PSUM") as scp,
                tc.tile_pool(name="avp", bufs=1, space="PSUM") as avp,
                tc.tile_pool(name="expp", bufs=3) as exp_pool,
                tc.tile_pool(name="recp", bufs=4) as rec_pool,
            ):
                for h in range(H):
                    ht, ho = h // 2, (h % 2) * 64
                    for s in range(4):
                        G = sched[s] // 4
                        av = [avp.tile([128, 65], F32, tag=f"av{q2}", name=f"av{q2}")
                              for q2 in range(2)]
                        for g in range(G):
                            sc = scp.tile([128, 1024], F32, tag="sc")
                            for jj in range(4):
                                kb = 4 * g + jj
                                nc.tensor.matmul(
                                    sc[:, jj * 256:(jj + 1) * 256],
                                    kT[ht][ho:ho + 64, kb * 128:(kb + 1) * 128],
                                    qT[ht][ho:ho + 64, s * 256:(s + 1) * 256],
                                    start=True, stop=True)
                            if masking and g == G - 1:
                                nc.vector.tensor_add(sc[:], sc[:], mask_sb[s][:])
                            ex = exp_pool.tile([128, 1024], BF16, tag="ex")
                            nc.scalar.activation(
                                ex[:], sc[:],
                                mybir.ActivationFunctionType.Exp, scale=0.125)
                            for jj in range(4):
                                for q2 in range(2):
                                    nc.tensor.matmul(
                                        av[q2][:],
                                        ex[:, jj * 256 + q2 * 128:
                                           jj * 256 + q2 * 128 + 128],
                                        v_sb[4 * g + jj][:, 65 * h:65 * h + 65],
                                        start=(g == 0 and jj == 0),
                                        stop=(g == G - 1 and jj == 3))
                        for q2 in range(2):
                            rec = rec_pool.tile([128, 1], F32, tag="rec")
                            nc.vector.reciprocal(rec[:], av[q2][:, 64:65])
                            nc.vector.tensor_scalar_mul(
                                attn[2 * s + q2][:, 64 * h:64 * h + 64],
                                av[q2][:, 0:64], rec[:])

            # ---- output projection ----
            with (
                tc.tile_pool(name="ostage", bufs=1) as osp,
                tc.tile_pool(name="outb", bufs=2) as obp,
                tc.tile_pool(name="psO", bufs=2, space="PSUM") as psO,
            ):
                attnT = [osp.tile([128, QROWS], BF16, tag=f"attnT{j}", name=f"attnT{j}")
                         for j in range(8)]
                for t in range(QROWS // 128):
                    for j in range(8):
                        nc.sync.dma_start_transpose(
                            attnT[j][:, t * 128:(t + 1) * 128],
                            attn[t][:, j * 128:(j + 1) * 128])
                w_sb = [osp.tile([128, DM], BF16, tag=f"wo{j}", name=f"wo{j}") for j in range(8)]
                for j in range(8):
                    nc.sync.dma_start(w_sb[j][:], wot[j * 128:(j + 1) * 128, :])
                for t in range(QROWS // 128):
                    ot = obp.tile([128, DM], F32, tag="ot")
                    for c in range(2):
                        p = psO.tile([128, 512], F32, tag="po")
                        for j in range(8):
                            nc.tensor.matmul(
                                p[:], attnT[j][:, t * 128:(t + 1) * 128],
                                w_sb[j][:, c * 512:(c + 1) * 512],
                                start=(j == 0), stop=False)
                        nc.tensor.matmul(
                            p[:], ones_sb[:], bo_sb[:, c * 512:(c + 1) * 512],
                            start=False, stop=True)
                        nc.vector.tensor_copy(ot[:, c * 512:(c + 1) * 512], p[:])
                    nc.sync.dma_start(out[t * 128:(t + 1) * 128, :], ot[:])

    _split_excess_waits(nc)
    return nc


def _build_masks(half: int) -> np.ndarray:
    """Additive mask for the LAST 4-kb group of each slot: [4, 128, 1024],
    free dim = kb_local*256 + dq."""
    m = np.zeros((4, 128, 1024), np.float32)
    dk = np.arange(128)[:, None]
    dq = np.arange(256)[None, :]
    for s in range(4):
        L = SCHED[s]
        g = GMAP[half][s]
        for jj in range(4):
            kb = L - 4 + jj
            kg = kb * 128 + dk
            qg = g * 256 + dq
            m[s, :, jj * 256:(jj + 1) * 256] = np.where(kg <= qg, 0.0, MASK_NEG)
    return m


_CACHE = {}


def kernel(query, key, value, Wq, bq, Wk, bk, Wv, bv, Wo, bo, masking):
    query = np.asarray(query, np.float32)
    key = np.asarray(key, np.float32)
    value = np.asarray(value, np.float32)
    masking = bool(int(np.asarray(masking)))

    bf = ml_dtypes.bfloat16
    wqt = np.ascontiguousarray(np.asarray(Wq, np.float32).T).astype(bf)
    wkt = np.ascontiguousarray(np.asarray(Wk, np.float32).T).astype(bf)
    wvt = np.ascontiguousarray(np.asarray(Wv, np.float32).T).astype(bf)
    wot = np.ascontiguousarray(np.asarray(Wo, np.float32).T).astype(bf)
    bq2 = np.ascontiguousarray(np.asarray(bq, np.float32).reshape(8, 128).T)
    bk2 = np.ascontiguousarray(np.asarray(bk, np.float32).reshape(8, 128).T)
    bvr = np.asarray(bv, np.float32).reshape(1, DM).astype(bf)
    bor = np.asarray(bo, np.float32).reshape(1, DM).astype(bf)

    if masking not in _CACHE:
        _CACHE[masking] = build_mha(masking)
    nc = _CACHE[masking]
    in_maps = make_in_maps(query, key, value, wqt, wkt, wvt, wot,
                           bq2, bk2, bvr, bor, masking)
    res = run_bass_kernel_spmd(nc, in_maps, list(range(N_CORES)))
    return gather_out([r["out"] for r in res.results], masking)


def make_in_maps(query, key, value, wqt, wkt, wvt, wot, bq2, bk2, bvr, bor,
                 masking):
    in_maps = []
    for c in range(N_CORES):
        b, half = c // 2, c % 2
        gmap = GMAP[half] if masking else (
            (0, 1, 2, 3) if half == 0 else (4, 5, 6, 7))
        qch = query[b].reshape(NCH, QCH, DM)
        q_sh = np.ascontiguousarray(
            np.concatenate([qch[g] for g in gmap], axis=0))
        in_maps.append({
            "q_in": q_sh, "k_in": key[b], "v_in": value[b],
            "wqt": wqt, "wkt": wkt, "wvt": wvt, "wot": wot,
            "bq2": bq2, "bk2": bk2, "bvr": bvr, "bor": bor,
            "msk": _build_masks(half) if masking else
                   np.zeros((4, 128, 1024), np.float32),
        })

    return in_maps


def gather_out(core_outs, masking):
    out = np.empty((B, S, DM), np.float32)
    for c in range(N_CORES):
        b, half = c // 2, c % 2
        gmap = GMAP[half] if masking else (
            (0, 1, 2, 3) if half == 0 else (4, 5, 6, 7))
        o = np.asarray(core_outs[c]).reshape(4, QCH, DM)
        for s, g in enumerate(gmap):
            out[b, g * QCH:(g + 1) * QCH, :] = o[s]
    return out


# revision 11
# speedup vs baseline: 74.3691x; 74.3691x over previous
"""Multi-head causal attention (B=4, S=2048, H=16, D=64) on 8 trn2 cores.

Sharding: core c -> (batch b = c//2, half = c%2). Each core computes the
full K/V projections for its batch and attention + output projection for
a zigzag set of 4 query chunks of 256 rows ({0,3,4,7} or {1,2,5,6}) so
that the causal-attention work per core is identical in structure
(uniform SPMD program); per-core differences are carried entirely by the
input data (query rows, additive masks). No collectives needed: each
core owns disjoint output rows.
"""

import numpy as np
import ml_dtypes

import concourse.bass as bass
import concourse.mybir as mybir
from concourse.tile import TileContext
from concourse.bass_utils import run_bass_kernel_spmd

F32 = mybir.dt.float32
BF16 = mybir.dt.bfloat16

B, S, H, D = 4, 2048, 16, 64
DM = H * D           # 1024
QCH = 256            # query chunk rows
NCH = S // QCH       # 8 global chunks per batch
QROWS = S // 2       # query rows per core (1024)
KB = 128             # k block rows
SCHED = (4, 8, 12, 16)   # padded k-extent (in KB blocks) per local slot
GMAP = ((0, 3, 4, 7), (1, 2, 5, 6))  # global chunk per (half, slot)
MASK_NEG = -30000.0

N_CORES = 8


def _split_excess_waits(nc):
    """walrus on this stack accepts at most ONE semaphore wait per
    instruction; Tile emits more on drains/branches/etc. Move excess
    waits onto preceding same-engine nops (semantically identical: the
    engine blocks on the nops first)."""
    for f in nc.m.functions:
        for bb in f.blocks:
            new_instrs = []
            for ins in bb.instructions:
                si = ins.sync_info
                if si is not None and si.on_wait is not None and len(si.on_wait) > 1:
                    waits = list(si.on_wait)
                    extra, keep = waits[:-1], waits[-1:]
                    for i, w in enumerate(extra):
                        new_instrs.append(mybir.InstNoOp(
                            name=f"{ins.name}-ws{i}", engine=ins.engine,
                            ins=[], outs=[],
                            sync_info=mybir.SyncInfo(on_wait=[w], on_update=[])))
                    ins.sync_info = mybir.SyncInfo(on_wait=keep,
                                                   on_update=list(si.on_update))
                new_instrs.append(ins)
            bb.instructions[:] = new_instrs


def _load_transposed(nc, pool, src_dram, rows, name, do_trans=True):
    """Load src [rows, DM] f32 from DRAM -> bf16 transposed SBUF tiles:
    returns 8 tiles [128, rows] (tile j = dm rows 128j..128j+127)."""
    tts = [pool.tile([128, rows], BF16, tag=f"{name}T{j}", name=f"{name}T{j}") for j in range(8)]
    n_rt = rows // 128
    bfs = []
    for st in range(n_rt):
        raw = pool.tile([128, DM], F32, tag=f"{name}_raw", bufs=2)
        eng = nc.sync if st % 2 == 0 else nc.scalar
        eng.dma_start(raw[:], src_dram[st * 128:(st + 1) * 128, :])
        bfv = pool.tile([128, DM], BF16, tag=f"{name}_bf{st}", name=f"{name}_bf{st}")
        if do_trans:
            nc.vector.tensor_copy(bfv[:], raw[:])
        bfs.append(bfv)
    # all transposes back-to-back: minimizes DMA xbar copy<->transpose
    # mode transitions, which serialize the DMA queues
    if do_trans:
        for st in range(n_rt):
            for j in range(8):
                eng = nc.sync if j % 2 == 0 else nc.scalar
                eng.dma_start_transpose(
                    tts[j][:, st * 128:(st + 1) * 128],
                    bfs[st][:, j * 128:(j + 1) * 128])
    return tts


def build_mha(masking: bool, repeat: int = 1, do_attn: bool = True,
              do_outproj: bool = True, do_proj: bool = True,
              do_trans: bool = True):
    nc = bass.Bass()

    q_in = nc.dram_tensor("q_in", [QROWS, DM], F32, kind="ExternalInput")
    k_in = nc.dram_tensor("k_in", [S, DM], F32, kind="ExternalInput")
    v_in = nc.dram_tensor("v_in", [S, DM], F32, kind="ExternalInput")
    wqt = nc.dram_tensor("wqt", [DM, DM], BF16, kind="ExternalInput")
    wkt = nc.dram_tensor("wkt", [DM, DM], BF16, kind="ExternalInput")
    wvt = nc.dram_tensor("wvt", [DM, DM], BF16, kind="ExternalInput")
    wot = nc.dram_tensor("wot", [DM, DM], BF16, kind="ExternalInput")
    bq2 = nc.dram_tensor("bq2", [128, 8], F32, kind="ExternalInput")
    bk2 = nc.dram_tensor("bk2", [128, 8], F32, kind="ExternalInput")
    bvr = nc.dram_tensor("bvr", [1, DM], BF16, kind="ExternalInput")
    bor = nc.dram_tensor("bor", [1, DM], BF16, kind="ExternalInput")
    msk = nc.dram_tensor("msk", [4, 128, 1024], F32, kind="ExternalInput")
    out = nc.dram_tensor("out", [QROWS, DM], F32, kind="ExternalOutput")

    sched = SCHED if masking else (16, 16, 16, 16)

    for _rep in range(repeat):
      with TileContext(nc) as tc:
        with (
            tc.tile_pool(name="persist", bufs=1) as pp,
            tc.tile_pool(name="psA", bufs=2, space="PSUM") as psA,
        ):
            # ---- constants ----
            bq_sb = pp.tile([128, 8], F32, tag="bq")
            bk_sb = pp.tile([128, 8], F32, tag="bk")
            nc.sync.dma_start(bq_sb[:], bq2[:])
            nc.sync.dma_start(bk_sb[:], bk2[:])
            bv_sb = pp.tile([1, DM], BF16, tag="bv")
            nc.sync.dma_start(bv_sb[:], bvr[:])
            bo_sb = pp.tile([1, DM], BF16, tag="bo")
            nc.sync.dma_start(bo_sb[:], bor[:])
            ones_sb = pp.tile([1, 128], BF16, tag="ones")
            nc.vector.memset(ones_sb[:], 1.0)
            mask_sb = []
            if masking:
                for s in range(4):
                    mt = pp.tile([128, 1024], F32, tag=f"msk{s}")
                    nc.sync.dma_start(mt[:], msk[s])
                    mask_sb.append(mt)

            # ---- persistent activation storage ----
            kT = [pp.tile([128, S], BF16, tag=f"kT{j}", name=f"kT{j}") for j in range(8)]
            qT = [pp.tile([128, QROWS], BF16, tag=f"qT{j}", name=f"qT{j}") for j in range(8)]
            v_sb = [pp.tile([128, H * (D + 1)], BF16, tag=f"v{t}", name=f"v{t}")
                    for t in range(S // 128)]

            # ---- K projection ----
            if do_proj:
              with tc.tile_pool(name="kstage", bufs=1) as ksp:
                keyT = _load_transposed(nc, ksp, k_in, S, "key", do_trans)
                w_sb = [ksp.tile([128, DM], BF16, tag=f"wk{j}", name=f"wk{j}") for j in range(8)]
                for j in range(8):
                    nc.sync.dma_start(w_sb[j][:], wkt[j * 128:(j + 1) * 128, :])
                for i in range(8):
                    for sc in range(4):
                        p = psA.tile([128, 512], F32, tag="proj")
                        for j in range(8):
                            nc.tensor.matmul(
                                p[:], w_sb[j][:, i * 128:(i + 1) * 128],
                                keyT[j][:, sc * 512:(sc + 1) * 512],
                                start=(j == 0), stop=(j == 7))
                        nc.vector.tensor_scalar_add(
                            kT[i][:, sc * 512:(sc + 1) * 512], p[:],
                            bk_sb[:, i:i + 1])

            # ---- V projection ----
            if do_proj:
              with tc.tile_pool(name="vstage", bufs=1) as vsp:
                valT = _load_transposed(nc, vsp, v_in, S, "val", do_trans)
                w_sb = [vsp.tile([128, DM], BF16, tag=f"wv{j}", name=f"wv{j}") for j in range(8)]
                for j in range(8):
                    nc.sync.dma_start(w_sb[j][:], wvt[j * 128:(j + 1) * 128, :])
                for st in range(S // 128):
                    v3 = v_sb[st].rearrange("p (h x) -> p h x", x=D + 1)
                    nc.vector.memset(v3[:, :, 64:65], 1.0)
                    for c in range(2):
                        p = psA.tile([128, 512], F32, tag="proj")
                        for j in range(8):
                            nc.tensor.matmul(
                                p[:], valT[j][:, st * 128:(st + 1) * 128],
                                w_sb[j][:, c * 512:(c + 1) * 512],
                                start=(j == 0), stop=False)
                        nc.tensor.matmul(
                            p[:], ones_sb[:], bv_sb[:, c * 512:(c + 1) * 512],
                            start=False, stop=True)
                        nc.vector.tensor_copy(
                            v3[:, c * 8:(c + 1) * 8, 0:64], p[:])

            # ---- Q projection ----
            if do_proj:
              with tc.tile_pool(name="qstage", bufs=1) as qsp:
                quT = _load_transposed(nc, qsp, q_in, QROWS, "qu", do_trans)
                w_sb = [qsp.tile([128, DM], BF16, tag=f"wq{j}", name=f"wq{j}") for j in range(8)]
                for j in range(8):
                    nc.sync.dma_start(w_sb[j][:], wqt[j * 128:(j + 1) * 128, :])
                for i in range(8):
                    for sc in range(2):
                        p = psA.tile([128, 512], F32, tag="proj")
                        for j in range(8):
                            nc.tensor.matmul(
                                p[:], w_sb[j][:, i * 128:(i + 1) * 128],
                                quT[j][:, sc * 512:(sc + 1) * 512],
                                start=(j == 0), stop=(j == 7))
                        nc.vector.tensor_scalar_add(
                            qT[i][:, sc * 512:(sc + 1) * 512], p[:],
                            bq_sb[:, i:i + 1])

            # ---- attention ----
            attn = [pp.tile([128, DM], BF16, tag=f"attn{t}", name=f"attn{t}")
                    for t in range(QROWS // 128)]
            if do_attn:
              with (
                tc.tile_pool(name="scores", bufs=2, space="PSUM") as scp,
                tc.tile_pool(name="avp", bufs=1, space="PSUM") as avp,
                tc.tile_pool(name="expp", bufs=3) as exp_pool,
                tc.tile_pool(name="recp", bufs=4) as rec_pool,
            ):
                for h in range(H):
                    ht, ho = h // 2, (h % 2) * 64
                    for s in range(4):
                        G = sched[s] // 4
                        av = [avp.tile([128, 65], F32, tag=f"av{q2}", name=f"av{q2}")
                              for q2 in range(2)]
                        for g in range(G):
                            sc = scp.tile([128, 1024], F32, tag="sc")
                            for jj in range(4):
                                kb = 4 * g + jj
                                nc.tensor.matmul(
                                    sc[:, jj * 256:(jj + 1) * 256],
                                    kT[ht][ho:ho + 64, kb * 128:(kb + 1) * 128],
                                    qT[ht][ho:ho + 64, s * 256:(s + 1) * 256],
                                    start=True, stop=True)
                            if masking and g == G - 1:
                                nc.vector.tensor_add(sc[:], sc[:], mask_sb[s][:])
                            ex = exp_pool.tile([128, 1024], BF16, tag="ex")
                            nc.scalar.activation(
                                ex[:], sc[:],
                                mybir.ActivationFunctionType.Exp, scale=0.125)
                            for jj in range(4):
                                for q2 in range(2):
                                    nc.tensor.matmul(
                                        av[q2][:],
                                        ex[:, jj * 256 + q2 * 128:
                                           jj * 256 + q2 * 128 + 128],
                                        v_sb[4 * g + jj][:, 65 * h:65 * h + 65],
                                        start=(g == 0 and jj == 0),
                                        stop=(g == G - 1 and jj == 3))
                        for q2 in range(2):
                            rec = rec_pool.tile([128, 1], F32, tag="rec")
                            nc.vector.reciprocal(rec[:], av[q2][:, 64:65])
                            nc.vector.tensor_scalar_mul(
                                attn[2 * s + q2][:, 64 * h:64 * h + 64],
                                av[q2][:, 0:64], rec[:])

            # ---- output projection ----
            if do_outproj:
              with (
                tc.tile_pool(name="ostage", bufs=1) as osp,
                tc.tile_pool(name="outb", bufs=2) as obp,
                tc.tile_pool(name="psO", bufs=2, space="PSUM") as psO,
            ):
                attnT = [osp.tile([128, QROWS], BF16, tag=f"attnT{j}", name=f"attnT{j}")
                         for j in range(8)]
                for t in range(QROWS // 128):
                    for j in range(8):
                        nc.sync.dma_start_transpose(
                            attnT[j][:, t * 128:(t + 1) * 128],
                            attn[t][:, j * 128:(j + 1) * 128])
                w_sb = [osp.tile([128, DM], BF16, tag=f"wo{j}", name=f"wo{j}") for j in range(8)]
                for j in range(8):
                    nc.sync.dma_start(w_sb[j][:], wot[j * 128:(j + 1) * 128, :])
                for t in range(QROWS // 128):
                    ot = obp.tile([128, DM], F32, tag="ot")
                    for c in range(2):
                        p = psO.tile([128, 512], F32, tag="po")
                        for j in range(8):
                            nc.tensor.matmul(
                                p[:], attnT[j][:, t * 128:(t + 1) * 128],
                                w_sb[j][:, c * 512:(c + 1) * 512],
                                start=(j == 0), stop=False)
                        nc.tensor.matmul(
                            p[:], ones_sb[:], bo_sb[:, c * 512:(c + 1) * 512],
                            start=False, stop=True)
                        nc.vector.tensor_copy(ot[:, c * 512:(c + 1) * 512], p[:])
                    nc.sync.dma_start(out[t * 128:(t + 1) * 128, :], ot[:])

    _split_excess_waits(nc)
    return nc


def _build_masks(half: int) -> np.ndarray:
    """Additive mask for the LAST 4-kb group of each slot: [4, 128, 1024],
    free dim = kb_local*256 + dq."""
    m = np.zeros((4, 128, 1024), np.float32)
    dk = np.arange(128)[:, None]
    dq = np.arange(256)[None, :]
    for s in range(4):
        L = SCHED[s]
        g = GMAP[half][s]
        for jj in range(4):
            kb = L - 4 + jj
            kg = kb * 128 + dk
            qg = g * 256 + dq
            m[s, :, jj * 256:(jj + 1) * 256] = np.where(kg <= qg, 0.0, MASK_NEG)
    return m


_CACHE = {}


def kernel(query, key, value, Wq, bq, Wk, bk, Wv, bv, Wo, bo, masking):
    query = np.asarray(query, np.float32)
    key = np.asarray(key, np.float32)
    value = np.asarray(value, np.float32)
    masking = bool(int(np.asarray(masking)))

    bf = ml_dtypes.bfloat16
    wqt = np.ascontiguousarray(np.asarray(Wq, np.float32).T).astype(bf)
    wkt = np.ascontiguousarray(np.asarray(Wk, np.float32).T).astype(bf)
    wvt = np.ascontiguousarray(np.asarray(Wv, np.float32).T).astype(bf)
    wot = np.ascontiguousarray(np.asarray(Wo, np.float32).T).astype(bf)
    bq2 = np.ascontiguousarray(np.asarray(bq, np.float32).reshape(8, 128).T)
    bk2 = np.ascontiguousarray(np.asarray(bk, np.float32).reshape(8, 128).T)
    bvr = np.asarray(bv, np.float32).reshape(1, DM).astype(bf)
    bor = np.asarray(bo, np.float32).reshape(1, DM).astype(bf)

    if masking not in _CACHE:
        _CACHE[masking] = build_mha(masking)
    nc = _CACHE[masking]
    in_maps = make_in_maps(query, key, value, wqt, wkt, wvt, wot,
                           bq2, bk2, bvr, bor, masking)
    res = run_bass_kernel_spmd(nc, in_maps, list(range(N_CORES)))
    return gather_out([r["out"] for r in res.results], masking)


def make_in_maps(query, key, value, wqt, wkt, wvt, wot, bq2, bk2, bvr, bor,
                 masking):
    in_maps = []
    for c in range(N_CORES):
        b, half = c // 2, c % 2
        gmap = GMAP[half] if masking else (
            (0, 1, 2, 3) if half == 0 else (4, 5, 6, 7))
        qch = query[b].reshape(NCH, QCH, DM)
        q_sh = np.ascontiguousarray(
            np.concatenate([qch[g] for g in gmap], axis=0))
        in_maps.append({
            "q_in": q_sh, "k_in": key[b], "v_in": value[b],
            "wqt": wqt, "wkt": wkt, "wvt": wvt, "wot": wot,
            "bq2": bq2, "bk2": bk2, "bvr": bvr, "bor": bor,
            "msk": _build_masks(half) if masking else
                   np.zeros((4, 128, 1024), np.float32),
        })

    return in_maps


def gather_out(core_outs, masking):
    out = np.empty((B, S, DM), np.float32)
    for c in range(N_CORES):
        b, half = c // 2, c % 2
        gmap = GMAP[half] if masking else (
            (0, 1, 2, 3) if half == 0 else (4, 5, 6, 7))
        o = np.asarray(core_outs[c]).reshape(4, QCH, DM)
        for s, g in enumerate(gmap):
            out[b, g * QCH:(g + 1) * QCH, :] = o[s]
    return out
